# revision 33
# baseline (speedup 1.0000x reference)
"""Trainium2 Bass kernel for segment-wise Conv1d + ReLU + BatchNorm1d.

Reference computation (nn_ConvSeg):
  - x_all [32768, 256] fp32, segment_key [32768] sorted ids (<= 8 segments)
  - per-segment Conv1d (kernel K=9, zero padding 4 at segment boundaries)
  - ReLU, then BatchNorm1d over all tokens (training stats, biased var)

Strategy:
  - Host inserts 4 zero rows at each segment boundary -> the ragged
    per-segment conv becomes ONE dense conv over the gapped sequence.
  - The gapped sequence (8*4104 positions) is split into 8 equal chunks
    (one per NeuronCore) with a 4-position halo on each side.
  - Data is transposed to [d, position] so each conv tap is a shifted
    column window of the same SBUF tile: conv = sum over (tap, d-chunk) of
    128x128 bf16 matmuls accumulated in fp32 PSUM ([d_out-chunk, pos]).
    bf16 inputs halve input DMA bytes at ~2.7e-3 rel err (tolerance 2e-2).
  - Blocks are 228 positions: measured on this hardware, matmuls with
    free dim <= ~228 stream ~2.6 cols/ns vs ~1.2 above ~256 - a 2x cliff
    (measured via pure-matmul count-differencing probes).
  - A few matmuls on a scratch tile run during the input-DMA head so the
    PE activity monitor un-throttles the clock before the real matmuls.
  - ScalarE fuses bias + ReLU from PSUM and accumulates per-block column
    sums (accum_out); a second ScalarE pass accumulates sums of squares.
    Results DMA out (bf16) per ~900-column group as soon as ready,
    overlapping the remaining matmuls; raw per-block sums ship last as a
    tiny [128, 4*NB] tensor.
  - The BatchNorm reduction across cores and the per-channel affine fold
    into the host-side unshard: the host subtracts the gap columns'
    contribution from the raw sums, reduces across the 8 cores, and
    applies y*scale+shift while reassembling [32768, 256]. No collective
    (the emulated-NRT AllReduce costs ~1 ms here), no second device pass.
"""

import numpy as np
import ml_dtypes

import concourse.bacc as bacc
import concourse.mybir as mybir
from concourse import tile
from concourse.bass_utils import run_bass_kernel_spmd

F32 = mybir.dt.float32
BF16 = mybir.dt.bfloat16
AF = mybir.ActivationFunctionType

N = 32768
D = 256  # d_in == d_out == 256
K = 9
PAD = K // 2
EPS = 1e-5

NCORES = 8
NB = 18  # matmul blocks per core
BS = 228  # positions per block (lives in the PE's fast free-dim regime)
L = NB * BS  # 4104 gapped positions per core
LH = L + 2 * PAD  # input columns incl. halo
GAP = 4  # zero rows inserted at each segment boundary (>= PAD)

# out-DMA column groups and x-DMA chunks (all boundaries are multiples of
# every supported block size)
OUTG = [(0, 912), (912, 1824), (1824, 2736), (2736, 3648), (3648, L)]
XCH = [(0, 464), (456, 920), (912, 2288), (2280, LH)]
# columns >= STAT_HI get their BN sums computed on the host (from the
# returned y) instead of on-device, so the stats DMA never sits on the
# critical tail - it only depends on the second-to-last output group
STAT_HI = 3648

_PROGRAM_CACHE: dict = {}


def build_program(repeat: int = 1, warm: int = 8, nb: int = None,
                  bs: int = None, out_bf16: bool = True):
    """Build + compile the SPMD Bass program (identical on all 8 cores)."""
    nb = NB if nb is None else nb
    bs = BS if bs is None else bs
    assert nb * bs == L
    ydt = BF16 if out_bf16 else F32
    nc = bacc.Bacc(
        "TRN2", target_bir_lowering=False, debug=False, num_devices=NCORES
    )

    nst = STAT_HI // bs  # blocks with on-device stats (per oc half)
    x_d = nc.declare_dram_parameter("x", [2, 128, LH], BF16, isOutput=False)
    w_d = nc.declare_dram_parameter("w", [2, 128, K * D], BF16, isOutput=False)
    b2_d = nc.declare_dram_parameter("b2", [128, 2], F32, isOutput=False)
    out_d = nc.declare_dram_parameter("out", [D, L], ydt, isOutput=True)
    st_d = nc.declare_dram_parameter("st", [128, 4 * nst], F32, isOutput=True)

    with tile.TileContext(nc) as tc:
        with (
            tc.tile_pool(name="const", bufs=1) as const,
            tc.tile_pool(name="ypool", bufs=1) as ypool,
            tc.tile_pool(name="psum", bufs=4, space="PSUM") as psum,
            tc.tile_pool(name="pswarm", bufs=1, space="PSUM") as pswarm,
            tc.tile_pool(name="work", bufs=2) as work,
            tc.tile_pool(name="stats", bufs=1) as stats,
        ):
            xt = [const.tile([128, LH], BF16, tag=f"xt{dc}", name=f"xt{dc}")
                  for dc in range(2)]
            wt = [const.tile([128, K * D], BF16, tag=f"wt{dc}", name=f"wt{dc}")
                  for dc in range(2)]
            b2t = const.tile([128, 2], F32)
            # scratch warmup operand: never written, contents irrelevant
            wz = const.tile([128, 464], BF16, tag="wz", name="wz")
            ybig = ypool.tile([128, 2 * L], ydt)
            # per-block raw sums: cols [0,2nst) = sum(y), rest = sum(y^2)
            stq = stats.tile([128, 4 * nst], F32)

            if warm:
                nc.gpsimd.memset(wz[:], 0.0)

            for _ in range(repeat):
                # --- PE warmup: no data deps, runs during the DMA head so
                # the activity monitor un-throttles the clock ---
                if warm:
                    psw = pswarm.tile([128, min(bs, 464)], F32, tag="psw")
                    for _ in range(warm):
                        nc.tensor.matmul(
                            psw[:], wz[:, 0:128], wz[:, 0 : min(bs, 464)],
                            start=True, stop=True,
                        )

                # --- input DMAs, ordered to match PE consumption times ---
                for dc in range(2):  # first x chunk
                    lo, hi = XCH[0]
                    nc.sync.dma_start(xt[dc][:, lo:hi], x_d[dc, :, lo:hi])
                for dc in range(2):  # tap k=0 weights
                    nc.sync.dma_start(wt[dc][:, 0:D], w_d[dc, :, 0:D])
                # remaining weights, dc-major halves so arrival order
                # matches the first group's dc-major consumption order
                for dc in range(2):
                    nc.sync.dma_start(
                        wt[dc][:, D : 5 * D], w_d[dc, :, D : 5 * D]
                    )
                    nc.sync.dma_start(wt[dc][:, 5 * D :], w_d[dc, :, 5 * D :])
                for dc in range(2):  # second x chunk
                    lo, hi = XCH[1]
                    nc.sync.dma_start(xt[dc][:, lo:hi], x_d[dc, :, lo:hi])
                nc.sync.dma_start(b2t[:], b2_d[:])  # needed by first relu
                for ch in range(2, len(XCH)):
                    lo, hi = XCH[ch]
                    for dc in range(2):
                        nc.sync.dma_start(xt[dc][:, lo:hi], x_d[dc, :, lo:hi])

                # --- conv + relu(+bias) + raw stats + streaming out-DMA ---
                for glo, ghi in OUTG:
                    for b in range(glo // bs, ghi // bs):
                        for oc in range(2):
                            ps = psum.tile([128, bs], F32, tag="ps")
                            # dc-major so the dc=0 taps can run while the
                            # dc=1 weight DMA is still in flight early on
                            for dc in range(2):
                                for k in range(K):
                                    nc.tensor.matmul(
                                        ps[:],
                                        wt[dc][
                                            :, k * D + oc * 128
                                            : k * D + oc * 128 + 128
                                        ],
                                        xt[dc][:, b * bs + k : b * bs + k + bs],
                                        start=(k == 0 and dc == 0),
                                        stop=(k == K - 1 and dc == 1),
                                    )
                            ysl = ybig[:, oc * L + b * bs : oc * L + (b + 1) * bs]
                            if b < nst:
                                j = oc * nst + b
                                # y = relu(conv + bias); accum_out = sum(y)
                                nc.scalar.activation(
                                    ysl, ps[:], AF.Relu,
                                    bias=b2t[:, oc : oc + 1], scale=1.0,
                                    accum_out=stq[:, j : j + 1],
                                )
                                # sum of squares via a second ScalarE pass
                                # (tensor_tensor_reduce crashes the device)
                                sq = work.tile([128, bs], F32, tag="sq")
                                nc.scalar.activation(
                                    sq[:], ysl, AF.Square, bias=0.0,
                                    scale=1.0,
                                    accum_out=stq[:, 2 * nst + j
                                                  : 2 * nst + j + 1],
                                )
                            else:
                                # host computes this tail region's stats
                                nc.scalar.activation(
                                    ysl, ps[:], AF.Relu,
                                    bias=b2t[:, oc : oc + 1], scale=1.0,
                                )
                    for oc in range(2):
                        # final group: second DMA via the Pool/SWDGE queue,
                        # whose descriptor-gen runs parallel to HWDGE's
                        eng = (
                            nc.gpsimd if (ghi == L and oc == 1) else nc.sync
                        )
                        eng.dma_start(
                            out_d[oc * 128 : (oc + 1) * 128, glo:ghi],
                            ybig[:, oc * L + glo : oc * L + ghi],
                        )
                    if ghi == STAT_HI:
                        # all on-device stats complete; ship them now so
                        # this DMA hides under the last compute group
                        nc.sync.dma_start(st_d[:], stq[:])

    nc.compile()
    return nc


def _get_program(repeat: int = 1):
    key = repeat
    if key not in _PROGRAM_CACHE:
        _PROGRAM_CACHE[key] = build_program(repeat)
    return _PROGRAM_CACHE[key]


def prepare_inputs(x_all, W, b, gamma, beta, segment_key):
    """Host-side sharding: gap insertion, transpose, per-core slicing.

    Returns (in_maps, aux); aux carries everything assemble_output needs.
    """
    x_all = np.ascontiguousarray(np.asarray(x_all, dtype=np.float32))
    W = np.asarray(W, dtype=np.float32)
    b = np.asarray(b, dtype=np.float32)
    gamma = np.asarray(gamma, dtype=np.float32)
    beta = np.asarray(beta, dtype=np.float32)
    seg = np.asarray(segment_key).reshape(-1)
    n = x_all.shape[0]
    assert n == N, f"kernel hardcodes N={N}, got {n}"

    # run-length segments of the sorted key
    change = np.flatnonzero(seg[1:] != seg[:-1]) + 1
    starts = np.concatenate(([0], change))
    ends = np.concatenate((change, [n]))
    nseg = len(starts)
    assert n + GAP * (nseg + 1) <= NCORES * L, "gapped sequence does not fit"

    # gapped position of each token
    tok_gpos = np.empty(n, dtype=np.int64)
    g = GAP
    for s, e in zip(starts, ends):
        tok_gpos[s:e] = g + np.arange(e - s)
        g += (e - s) + GAP

    # gapped, transposed input with halo: xg_t[:, PAD + gpos] = x_all[n]
    total = NCORES * L
    xg = np.zeros((total + 2 * PAD, D), dtype=np.float32)
    xg[PAD + tok_gpos] = x_all
    xg_t = np.ascontiguousarray(xg.T.astype(ml_dtypes.bfloat16))

    # weights: wmat[d, k*D + o] = W[o, d, k]
    wmat = W.transpose(1, 2, 0).reshape(D, K * D).astype(ml_dtypes.bfloat16)
    w_in = np.ascontiguousarray(wmat.reshape(2, 128, K * D))

    b2 = np.ascontiguousarray(np.stack([b[:128], b[128:]], axis=1))

    in_maps = []
    for c in range(NCORES):
        xc = np.ascontiguousarray(
            xg_t[:, c * L : c * L + LH].reshape(2, 128, LH)
        )
        in_maps.append({"x": xc, "w": w_in, "b2": b2})
    aux = {"tok_gpos": tok_gpos, "gamma": gamma, "beta": beta}
    return in_maps, aux


def assemble_output(results, aux):
    """Unshard + fold the BatchNorm affine.

    Device sums include the gap columns; subtract their contribution (from
    the very same f32 y values the device summed), reduce across cores,
    then apply y*scale + shift per channel while gathering.
    """
    tok_gpos = aux["tok_gpos"]
    gamma, beta = aux["gamma"], aux["beta"]
    core = tok_gpos // L
    loc = tok_gpos % L

    nst = STAT_HI // BS
    S = np.zeros(D, dtype=np.float64)
    Q = np.zeros(D, dtype=np.float64)
    for c in range(NCORES):
        st = results[c]["st"].astype(np.float64)
        S += np.concatenate(
            [st[:, 0:nst].sum(axis=1), st[:, nst : 2 * nst].sum(axis=1)]
        )
        Q += np.concatenate(
            [st[:, 2 * nst : 3 * nst].sum(axis=1), st[:, 3 * nst :].sum(axis=1)]
        )
    valid = np.zeros((NCORES, L), dtype=bool)
    valid[core, loc] = True
    for c in range(NCORES):
        # device summed cols [0, STAT_HI) incl. gaps: subtract the gaps
        gap_cols = np.flatnonzero(~valid[c][:STAT_HI])
        yg = results[c]["out"][:, gap_cols].astype(np.float64)
        S -= yg.sum(axis=1)
        Q -= (yg * yg).sum(axis=1)
        # cols [STAT_HI, L) have no device stats: add their valid columns
        tail_cols = STAT_HI + np.flatnonzero(valid[c][STAT_HI:])
        yt = results[c]["out"][:, tail_cols].astype(np.float64)
        S += yt.sum(axis=1)
        Q += (yt * yt).sum(axis=1)

    mean = S / N
    var = Q / N - mean * mean
    scale = gamma.astype(np.float64) / np.sqrt(var + EPS)
    shift = beta.astype(np.float64) - mean * scale
    scale32 = scale.astype(np.float32)
    shift32 = shift.astype(np.float32)

    out = np.empty((N, D), dtype=np.float32)
    for c in range(NCORES):
        sel = core == c
        yc = results[c]["out"][:, loc[sel]].T.astype(np.float32)
        out[sel] = yc * scale32 + shift32
    return out


def kernel(x_all, W, b, gamma, beta, segment_key):
    nc = _get_program()
    in_maps, aux = prepare_inputs(x_all, W, b, gamma, beta, segment_key)
    res = run_bass_kernel_spmd(nc, in_maps, list(range(NCORES)))
    return assemble_output(res.results, aux)


# revision 34
# speedup vs baseline: 1.0056x; 1.0056x over previous
"""Trainium2 Bass kernel for segment-wise Conv1d + ReLU + BatchNorm1d.

Reference computation (nn_ConvSeg):
  - x_all [32768, 256] fp32, segment_key [32768] sorted ids (<= 8 segments)
  - per-segment Conv1d (kernel K=9, zero padding 4 at segment boundaries)
  - ReLU, then BatchNorm1d over all tokens (training stats, biased var)

Strategy:
  - Host inserts 4 zero rows at each segment boundary -> the ragged
    per-segment conv becomes ONE dense conv over the gapped sequence.
  - The gapped sequence (8*4104 positions) is split into 8 equal chunks
    (one per NeuronCore) with a 4-position halo on each side.
  - Data is transposed to [d, position] so each conv tap is a shifted
    column window of the same SBUF tile: conv = sum over (tap, d-chunk) of
    128x128 bf16 matmuls accumulated in fp32 PSUM ([d_out-chunk, pos]).
    bf16 inputs halve input DMA bytes at ~2.7e-3 rel err (tolerance 2e-2).
  - Blocks are 228 positions: measured on this hardware, matmuls with
    free dim <= ~228 stream ~2.6 cols/ns vs ~1.2 above ~256 - a 2x cliff
    (measured via pure-matmul count-differencing probes).
  - A few matmuls on a scratch tile run during the input-DMA head so the
    PE activity monitor un-throttles the clock before the real matmuls.
  - ScalarE fuses bias + ReLU from PSUM and accumulates per-block column
    sums (accum_out); a second ScalarE pass accumulates sums of squares.
    Results DMA out (bf16) per ~900-column group as soon as ready,
    overlapping the remaining matmuls; raw per-block sums ship last as a
    tiny [128, 4*NB] tensor.
  - The BatchNorm reduction across cores and the per-channel affine fold
    into the host-side unshard: the host subtracts the gap columns'
    contribution from the raw sums, reduces across the 8 cores, and
    applies y*scale+shift while reassembling [32768, 256]. No collective
    (the emulated-NRT AllReduce costs ~1 ms here), no second device pass.
"""

import numpy as np
import ml_dtypes

import concourse.bacc as bacc
import concourse.mybir as mybir
from concourse import tile
from concourse.bass_utils import run_bass_kernel_spmd

F32 = mybir.dt.float32
BF16 = mybir.dt.bfloat16
AF = mybir.ActivationFunctionType

N = 32768
D = 256  # d_in == d_out == 256
K = 9
PAD = K // 2
EPS = 1e-5

NCORES = 8
NB = 18  # matmul blocks per core
BS = 228  # positions per block (lives in the PE's fast free-dim regime)
L = NB * BS  # 4104 gapped positions per core
LH = L + 2 * PAD  # input columns incl. halo
GAP = 4  # zero rows inserted at each segment boundary (>= PAD)

# out-DMA column groups and x-DMA chunks (all boundaries are multiples of
# every supported block size)
OUTG = [(0, 912), (912, 1824), (1824, 2736), (2736, 3648), (3648, L)]
XCH = [(0, 464), (456, 920), (912, 2288), (2280, LH)]
# columns >= STAT_HI get their BN sums computed on the host (from the
# returned y) instead of on-device, so the stats DMA never sits on the
# critical tail - it only depends on the second-to-last output group
STAT_HI = 3648

_PROGRAM_CACHE: dict = {}


def build_program(repeat: int = 1, warm: int = 8, nb: int = None,
                  bs: int = None, out_bf16: bool = True):
    """Build + compile the SPMD Bass program (identical on all 8 cores)."""
    nb = NB if nb is None else nb
    bs = BS if bs is None else bs
    assert nb * bs == L
    ydt = BF16 if out_bf16 else F32
    nc = bacc.Bacc(
        "TRN2", target_bir_lowering=False, debug=False, num_devices=NCORES
    )

    nst = STAT_HI // bs  # blocks with on-device stats (per oc half)
    x_d = nc.declare_dram_parameter("x", [2, 128, LH], BF16, isOutput=False)
    w_d = nc.declare_dram_parameter("w", [2, 128, K * D], BF16, isOutput=False)
    b2_d = nc.declare_dram_parameter("b2", [128, 2], F32, isOutput=False)
    out_d = nc.declare_dram_parameter("out", [D, L], ydt, isOutput=True)
    st_d = nc.declare_dram_parameter("st", [128, 4 * nst], F32, isOutput=True)

    with tile.TileContext(nc) as tc:
        with (
            tc.tile_pool(name="const", bufs=1) as const,
            tc.tile_pool(name="ypool", bufs=1) as ypool,
            tc.tile_pool(name="psum", bufs=4, space="PSUM") as psum,
            tc.tile_pool(name="pswarm", bufs=1, space="PSUM") as pswarm,
            tc.tile_pool(name="work", bufs=2) as work,
            tc.tile_pool(name="stats", bufs=1) as stats,
        ):
            xt = [const.tile([128, LH], BF16, tag=f"xt{dc}", name=f"xt{dc}")
                  for dc in range(2)]
            wt = [const.tile([128, K * D], BF16, tag=f"wt{dc}", name=f"wt{dc}")
                  for dc in range(2)]
            b2t = const.tile([128, 2], F32)
            # scratch warmup operand: never written, contents irrelevant
            wz = const.tile([128, 464], BF16, tag="wz", name="wz")
            ybig = ypool.tile([128, 2 * L], ydt)
            # per-block raw sums: cols [0,2nst) = sum(y), rest = sum(y^2)
            stq = stats.tile([128, 4 * nst], F32)

            if warm:
                nc.gpsimd.memset(wz[:], 0.0)

            for _ in range(repeat):
                # --- PE warmup: no data deps, runs during the DMA head so
                # the activity monitor un-throttles the clock ---
                if warm:
                    psw = pswarm.tile([128, min(bs, 464)], F32, tag="psw")
                    for _ in range(warm):
                        nc.tensor.matmul(
                            psw[:], wz[:, 0:128], wz[:, 0 : min(bs, 464)],
                            start=True, stop=True,
                        )

                # --- input DMAs, ordered to match PE consumption times ---
                for dc in range(2):  # first x chunk
                    lo, hi = XCH[0]
                    nc.sync.dma_start(xt[dc][:, lo:hi], x_d[dc, :, lo:hi])
                for dc in range(2):  # tap k=0 weights
                    nc.sync.dma_start(wt[dc][:, 0:D], w_d[dc, :, 0:D])
                # remaining weights, dc-major halves so arrival order
                # matches the first group's dc-major consumption order
                for dc in range(2):
                    nc.sync.dma_start(
                        wt[dc][:, D : 5 * D], w_d[dc, :, D : 5 * D]
                    )
                    nc.sync.dma_start(wt[dc][:, 5 * D :], w_d[dc, :, 5 * D :])
                for dc in range(2):  # second x chunk
                    lo, hi = XCH[1]
                    nc.sync.dma_start(xt[dc][:, lo:hi], x_d[dc, :, lo:hi])
                nc.sync.dma_start(b2t[:], b2_d[:])  # needed by first relu
                for ch in range(2, len(XCH)):
                    lo, hi = XCH[ch]
                    for dc in range(2):
                        nc.sync.dma_start(xt[dc][:, lo:hi], x_d[dc, :, lo:hi])

                # --- conv + relu(+bias) + raw stats + streaming out-DMA ---
                for glo, ghi in OUTG:
                    for b in range(glo // bs, ghi // bs):
                        for oc in range(2):
                            ps = psum.tile([128, bs], F32, tag="ps")
                            # dc-major so the dc=0 taps can run while the
                            # dc=1 weight DMA is still in flight early on
                            for dc in range(2):
                                for k in range(K):
                                    nc.tensor.matmul(
                                        ps[:],
                                        wt[dc][
                                            :, k * D + oc * 128
                                            : k * D + oc * 128 + 128
                                        ],
                                        xt[dc][:, b * bs + k : b * bs + k + bs],
                                        start=(k == 0 and dc == 0),
                                        stop=(k == K - 1 and dc == 1),
                                    )
                            ysl = ybig[:, oc * L + b * bs : oc * L + (b + 1) * bs]
                            if b < nst:
                                j = oc * nst + b
                                # y = relu(conv + bias); accum_out = sum(y)
                                nc.scalar.activation(
                                    ysl, ps[:], AF.Relu,
                                    bias=b2t[:, oc : oc + 1], scale=1.0,
                                    accum_out=stq[:, j : j + 1],
                                )
                                # sum of squares via a second ScalarE pass
                                # (tensor_tensor_reduce crashes the device)
                                sq = work.tile([128, bs], F32, tag="sq")
                                nc.scalar.activation(
                                    sq[:], ysl, AF.Square, bias=0.0,
                                    scale=1.0,
                                    accum_out=stq[:, 2 * nst + j
                                                  : 2 * nst + j + 1],
                                )
                            else:
                                # host computes this tail region's stats
                                nc.scalar.activation(
                                    ysl, ps[:], AF.Relu,
                                    bias=b2t[:, oc : oc + 1], scale=1.0,
                                )
                    for oc in range(2):
                        nc.sync.dma_start(
                            out_d[oc * 128 : (oc + 1) * 128, glo:ghi],
                            ybig[:, oc * L + glo : oc * L + ghi],
                        )
                    if ghi == STAT_HI:
                        # all on-device stats complete; ship them now so
                        # this DMA hides under the last compute group
                        nc.sync.dma_start(st_d[:], stq[:])

    nc.compile()
    return nc


def _get_program(repeat: int = 1):
    key = repeat
    if key not in _PROGRAM_CACHE:
        _PROGRAM_CACHE[key] = build_program(repeat)
    return _PROGRAM_CACHE[key]


def prepare_inputs(x_all, W, b, gamma, beta, segment_key):
    """Host-side sharding: gap insertion, transpose, per-core slicing.

    Returns (in_maps, aux); aux carries everything assemble_output needs.
    """
    x_all = np.ascontiguousarray(np.asarray(x_all, dtype=np.float32))
    W = np.asarray(W, dtype=np.float32)
    b = np.asarray(b, dtype=np.float32)
    gamma = np.asarray(gamma, dtype=np.float32)
    beta = np.asarray(beta, dtype=np.float32)
    seg = np.asarray(segment_key).reshape(-1)
    n = x_all.shape[0]
    assert n == N, f"kernel hardcodes N={N}, got {n}"

    # run-length segments of the sorted key
    change = np.flatnonzero(seg[1:] != seg[:-1]) + 1
    starts = np.concatenate(([0], change))
    ends = np.concatenate((change, [n]))
    nseg = len(starts)
    assert n + GAP * (nseg + 1) <= NCORES * L, "gapped sequence does not fit"

    # gapped position of each token
    tok_gpos = np.empty(n, dtype=np.int64)
    g = GAP
    for s, e in zip(starts, ends):
        tok_gpos[s:e] = g + np.arange(e - s)
        g += (e - s) + GAP

    # gapped, transposed input with halo: xg_t[:, PAD + gpos] = x_all[n]
    total = NCORES * L
    xg = np.zeros((total + 2 * PAD, D), dtype=np.float32)
    xg[PAD + tok_gpos] = x_all
    xg_t = np.ascontiguousarray(xg.T.astype(ml_dtypes.bfloat16))

    # weights: wmat[d, k*D + o] = W[o, d, k]
    wmat = W.transpose(1, 2, 0).reshape(D, K * D).astype(ml_dtypes.bfloat16)
    w_in = np.ascontiguousarray(wmat.reshape(2, 128, K * D))

    b2 = np.ascontiguousarray(np.stack([b[:128], b[128:]], axis=1))

    in_maps = []
    for c in range(NCORES):
        xc = np.ascontiguousarray(
            xg_t[:, c * L : c * L + LH].reshape(2, 128, LH)
        )
        in_maps.append({"x": xc, "w": w_in, "b2": b2})
    aux = {"tok_gpos": tok_gpos, "gamma": gamma, "beta": beta}
    return in_maps, aux


def assemble_output(results, aux):
    """Unshard + fold the BatchNorm affine.

    Device sums include the gap columns; subtract their contribution (from
    the very same f32 y values the device summed), reduce across cores,
    then apply y*scale + shift per channel while gathering.
    """
    tok_gpos = aux["tok_gpos"]
    gamma, beta = aux["gamma"], aux["beta"]
    core = tok_gpos // L
    loc = tok_gpos % L

    nst = STAT_HI // BS
    S = np.zeros(D, dtype=np.float64)
    Q = np.zeros(D, dtype=np.float64)
    for c in range(NCORES):
        st = results[c]["st"].astype(np.float64)
        S += np.concatenate(
            [st[:, 0:nst].sum(axis=1), st[:, nst : 2 * nst].sum(axis=1)]
        )
        Q += np.concatenate(
            [st[:, 2 * nst : 3 * nst].sum(axis=1), st[:, 3 * nst :].sum(axis=1)]
        )
    valid = np.zeros((NCORES, L), dtype=bool)
    valid[core, loc] = True
    for c in range(NCORES):
        # device summed cols [0, STAT_HI) incl. gaps: subtract the gaps
        gap_cols = np.flatnonzero(~valid[c][:STAT_HI])
        yg = results[c]["out"][:, gap_cols].astype(np.float64)
        S -= yg.sum(axis=1)
        Q -= (yg * yg).sum(axis=1)
        # cols [STAT_HI, L) have no device stats: add their valid columns
        tail_cols = STAT_HI + np.flatnonzero(valid[c][STAT_HI:])
        yt = results[c]["out"][:, tail_cols].astype(np.float64)
        S += yt.sum(axis=1)
        Q += (yt * yt).sum(axis=1)

    mean = S / N
    var = Q / N - mean * mean
    scale = gamma.astype(np.float64) / np.sqrt(var + EPS)
    shift = beta.astype(np.float64) - mean * scale
    scale32 = scale.astype(np.float32)
    shift32 = shift.astype(np.float32)

    out = np.empty((N, D), dtype=np.float32)
    for c in range(NCORES):
        sel = core == c
        yc = results[c]["out"][:, loc[sel]].T.astype(np.float32)
        out[sel] = yc * scale32 + shift32
    return out


def kernel(x_all, W, b, gamma, beta, segment_key):
    nc = _get_program()
    in_maps, aux = prepare_inputs(x_all, W, b, gamma, beta, segment_key)
    res = run_bass_kernel_spmd(nc, in_maps, list(range(NCORES)))
    return assemble_output(res.results, aux)


# revision 38
# speedup vs baseline: 1.0164x; 1.0108x over previous
"""Trainium2 Bass kernel for segment-wise Conv1d + ReLU + BatchNorm1d.

Reference computation (nn_ConvSeg):
  - x_all [32768, 256] fp32, segment_key [32768] sorted ids (<= 8 segments)
  - per-segment Conv1d (kernel K=9, zero padding 4 at segment boundaries)
  - ReLU, then BatchNorm1d over all tokens (training stats, biased var)

Strategy:
  - Host inserts 4 zero rows at each segment boundary -> the ragged
    per-segment conv becomes ONE dense conv over the gapped sequence.
  - The gapped sequence (8*4104 positions) is split into 8 equal chunks
    (one per NeuronCore) with a 4-position halo on each side.
  - Data is transposed to [d, position] so each conv tap is a shifted
    column window of the same SBUF tile: conv = sum over (tap, d-chunk) of
    128x128 bf16 matmuls accumulated in fp32 PSUM ([d_out-chunk, pos]).
    bf16 inputs halve input DMA bytes at ~2.7e-3 rel err (tolerance 2e-2).
  - Blocks are 228 positions: measured on this hardware, matmuls with
    free dim <= ~228 stream ~2.6 cols/ns vs ~1.2 above ~256 - a 2x cliff
    (measured via pure-matmul count-differencing probes).
  - A few matmuls on a scratch tile run during the input-DMA head so the
    PE activity monitor un-throttles the clock before the real matmuls.
  - ScalarE fuses bias + ReLU from PSUM and accumulates per-block column
    sums (accum_out); a second ScalarE pass accumulates sums of squares.
    Results DMA out (bf16) per ~900-column group as soon as ready,
    overlapping the remaining matmuls; raw per-block sums ship last as a
    tiny [128, 4*NB] tensor.
  - The BatchNorm reduction across cores and the per-channel affine fold
    into the host-side unshard: the host subtracts the gap columns'
    contribution from the raw sums, reduces across the 8 cores, and
    applies y*scale+shift while reassembling [32768, 256]. No collective
    (the emulated-NRT AllReduce costs ~1 ms here), no second device pass.
"""

import numpy as np
import ml_dtypes

import concourse.bacc as bacc
import concourse.mybir as mybir
from concourse import tile
from concourse.bass_utils import run_bass_kernel_spmd

F32 = mybir.dt.float32
BF16 = mybir.dt.bfloat16
AF = mybir.ActivationFunctionType

N = 32768
D = 256  # d_in == d_out == 256
K = 9
PAD = K // 2
EPS = 1e-5

NCORES = 8
NB = 18  # matmul blocks per core
BS = 228  # positions per block (lives in the PE's fast free-dim regime)
L = NB * BS  # 4104 gapped positions per core
LH = L + 2 * PAD  # input columns incl. halo
GAP = 4  # zero rows inserted at each segment boundary (>= PAD)

# out-DMA column groups and x-DMA chunks (all boundaries are multiples of
# every supported block size)
OUTG = [(0, 912), (912, 1824), (1824, 2736), (2736, 3648), (3648, L)]
XCH = [(0, 692), (684, 920), (912, 2288), (2280, LH)]
# columns >= STAT_HI get their BN sums computed on the host (from the
# returned y) instead of on-device, so the stats DMA never sits on the
# critical tail - it only depends on the second-to-last output group
STAT_HI = 3648

_PROGRAM_CACHE: dict = {}


def build_program(repeat: int = 1, warm: int = 8, nb: int = None,
                  bs: int = None, out_bf16: bool = True):
    """Build + compile the SPMD Bass program (identical on all 8 cores)."""
    nb = NB if nb is None else nb
    bs = BS if bs is None else bs
    assert nb * bs == L
    ydt = BF16 if out_bf16 else F32
    nc = bacc.Bacc(
        "TRN2", target_bir_lowering=False, debug=False, num_devices=NCORES
    )

    nst = STAT_HI // bs  # blocks with on-device stats (per oc half)
    x_d = nc.declare_dram_parameter("x", [2, 128, LH], BF16, isOutput=False)
    w_d = nc.declare_dram_parameter("w", [2, 128, K * D], BF16, isOutput=False)
    b2_d = nc.declare_dram_parameter("b2", [128, 2], F32, isOutput=False)
    out_d = nc.declare_dram_parameter("out", [D, L], ydt, isOutput=True)
    st_d = nc.declare_dram_parameter("st", [128, 4 * nst], F32, isOutput=True)

    with tile.TileContext(nc) as tc:
        with (
            tc.tile_pool(name="const", bufs=1) as const,
            tc.tile_pool(name="ypool", bufs=1) as ypool,
            tc.tile_pool(name="psum", bufs=6, space="PSUM") as psum,
            tc.tile_pool(name="pswarm", bufs=1, space="PSUM") as pswarm,
            tc.tile_pool(name="work", bufs=2) as work,
            tc.tile_pool(name="stats", bufs=1) as stats,
        ):
            xt = [const.tile([128, LH], BF16, tag=f"xt{dc}", name=f"xt{dc}")
                  for dc in range(2)]
            wt = [const.tile([128, K * D], BF16, tag=f"wt{dc}", name=f"wt{dc}")
                  for dc in range(2)]
            b2t = const.tile([128, 2], F32)
            # scratch warmup operand: never written, contents irrelevant
            wz = const.tile([128, 464], BF16, tag="wz", name="wz")
            ybig = ypool.tile([128, 2 * L], ydt)
            # per-block raw sums: cols [0,2nst) = sum(y), rest = sum(y^2)
            stq = stats.tile([128, 4 * nst], F32)

            if warm:
                nc.gpsimd.memset(wz[:], 0.0)

            for _ in range(repeat):
                # --- PE warmup: no data deps, runs during the DMA head so
                # the activity monitor un-throttles the clock ---
                if warm:
                    psw = pswarm.tile([128, min(bs, 464)], F32, tag="psw")
                    for _ in range(warm):
                        nc.tensor.matmul(
                            psw[:], wz[:, 0:128], wz[:, 0 : min(bs, 464)],
                            start=True, stop=True,
                        )

                # --- input DMAs, ordered to match PE consumption times ---
                for dc in range(2):  # tap k=0 weights
                    nc.sync.dma_start(wt[dc][:, 0:D], w_d[dc, :, 0:D])
                for dc in range(2):  # first x chunk
                    lo, hi = XCH[0]
                    nc.sync.dma_start(xt[dc][:, lo:hi], x_d[dc, :, lo:hi])
                # remaining weights, dc-major halves so arrival order
                # matches the first group's dc-major consumption order
                for dc in range(2):
                    nc.sync.dma_start(
                        wt[dc][:, D : 5 * D], w_d[dc, :, D : 5 * D]
                    )
                    nc.sync.dma_start(wt[dc][:, 5 * D :], w_d[dc, :, 5 * D :])
                for dc in range(2):  # second x chunk
                    lo, hi = XCH[1]
                    nc.sync.dma_start(xt[dc][:, lo:hi], x_d[dc, :, lo:hi])
                nc.sync.dma_start(b2t[:], b2_d[:])  # needed by first relu
                for ch in range(2, len(XCH)):
                    lo, hi = XCH[ch]
                    for dc in range(2):
                        nc.sync.dma_start(xt[dc][:, lo:hi], x_d[dc, :, lo:hi])

                # --- conv + relu(+bias) + raw stats + streaming out-DMA ---
                # k=0 pre-pass over the first blocks: needs only the first
                # x chunk + k0 weights, which land ~1.4us before the rest
                # of the weights - fills the PE's wait for that DMA
                nA = (XCH[0][1] - 2 * PAD) // bs
                pre = {}
                for b in range(nA):
                    for oc in range(2):
                        ps = psum.tile([128, bs], F32, tag="ps")
                        pre[(b, oc)] = ps
                        for dc in range(2):
                            nc.tensor.matmul(
                                ps[:],
                                wt[dc][:, oc * 128 : oc * 128 + 128],
                                xt[dc][:, b * bs : b * bs + bs],
                                start=(dc == 0), stop=False,
                            )
                for glo, ghi in OUTG:
                    for b in range(glo // bs, ghi // bs):
                        for oc in range(2):
                            if (b, oc) in pre:
                                ps = pre[(b, oc)]
                                k0 = 1
                            else:
                                ps = psum.tile([128, bs], F32, tag="ps")
                                k0 = 0
                            # dc-major so the dc=0 taps can run while the
                            # dc=1 weight DMA is still in flight early on
                            for dc in range(2):
                                for k in range(k0, K):
                                    nc.tensor.matmul(
                                        ps[:],
                                        wt[dc][
                                            :, k * D + oc * 128
                                            : k * D + oc * 128 + 128
                                        ],
                                        xt[dc][:, b * bs + k : b * bs + k + bs],
                                        start=(k0 == 0 and k == 0 and dc == 0),
                                        stop=(k == K - 1 and dc == 1),
                                    )
                            ysl = ybig[:, oc * L + b * bs : oc * L + (b + 1) * bs]
                            if b < nst:
                                j = oc * nst + b
                                # y = relu(conv + bias); accum_out = sum(y)
                                nc.scalar.activation(
                                    ysl, ps[:], AF.Relu,
                                    bias=b2t[:, oc : oc + 1], scale=1.0,
                                    accum_out=stq[:, j : j + 1],
                                )
                                # sum of squares via a second ScalarE pass
                                # (tensor_tensor_reduce crashes the device)
                                sq = work.tile([128, bs], F32, tag="sq")
                                nc.scalar.activation(
                                    sq[:], ysl, AF.Square, bias=0.0,
                                    scale=1.0,
                                    accum_out=stq[:, 2 * nst + j
                                                  : 2 * nst + j + 1],
                                )
                            else:
                                # host computes this tail region's stats
                                nc.scalar.activation(
                                    ysl, ps[:], AF.Relu,
                                    bias=b2t[:, oc : oc + 1], scale=1.0,
                                )
                    for oc in range(2):
                        nc.sync.dma_start(
                            out_d[oc * 128 : (oc + 1) * 128, glo:ghi],
                            ybig[:, oc * L + glo : oc * L + ghi],
                        )
                    if ghi == STAT_HI:
                        # all on-device stats complete; ship them now so
                        # this DMA hides under the last compute group
                        nc.sync.dma_start(st_d[:], stq[:])

    nc.compile()
    return nc


def _get_program(repeat: int = 1):
    key = repeat
    if key not in _PROGRAM_CACHE:
        _PROGRAM_CACHE[key] = build_program(repeat)
    return _PROGRAM_CACHE[key]


def prepare_inputs(x_all, W, b, gamma, beta, segment_key):
    """Host-side sharding: gap insertion, transpose, per-core slicing.

    Returns (in_maps, aux); aux carries everything assemble_output needs.
    """
    x_all = np.ascontiguousarray(np.asarray(x_all, dtype=np.float32))
    W = np.asarray(W, dtype=np.float32)
    b = np.asarray(b, dtype=np.float32)
    gamma = np.asarray(gamma, dtype=np.float32)
    beta = np.asarray(beta, dtype=np.float32)
    seg = np.asarray(segment_key).reshape(-1)
    n = x_all.shape[0]
    assert n == N, f"kernel hardcodes N={N}, got {n}"

    # run-length segments of the sorted key
    change = np.flatnonzero(seg[1:] != seg[:-1]) + 1
    starts = np.concatenate(([0], change))
    ends = np.concatenate((change, [n]))
    nseg = len(starts)
    assert n + GAP * (nseg + 1) <= NCORES * L, "gapped sequence does not fit"

    # gapped position of each token
    tok_gpos = np.empty(n, dtype=np.int64)
    g = GAP
    for s, e in zip(starts, ends):
        tok_gpos[s:e] = g + np.arange(e - s)
        g += (e - s) + GAP

    # gapped, transposed input with halo: xg_t[:, PAD + gpos] = x_all[n]
    total = NCORES * L
    xg = np.zeros((total + 2 * PAD, D), dtype=np.float32)
    xg[PAD + tok_gpos] = x_all
    xg_t = np.ascontiguousarray(xg.T.astype(ml_dtypes.bfloat16))

    # weights: wmat[d, k*D + o] = W[o, d, k]
    wmat = W.transpose(1, 2, 0).reshape(D, K * D).astype(ml_dtypes.bfloat16)
    w_in = np.ascontiguousarray(wmat.reshape(2, 128, K * D))

    b2 = np.ascontiguousarray(np.stack([b[:128], b[128:]], axis=1))

    in_maps = []
    for c in range(NCORES):
        xc = np.ascontiguousarray(
            xg_t[:, c * L : c * L + LH].reshape(2, 128, LH)
        )
        in_maps.append({"x": xc, "w": w_in, "b2": b2})
    aux = {"tok_gpos": tok_gpos, "gamma": gamma, "beta": beta}
    return in_maps, aux


def assemble_output(results, aux):
    """Unshard + fold the BatchNorm affine.

    Device sums include the gap columns; subtract their contribution (from
    the very same f32 y values the device summed), reduce across cores,
    then apply y*scale + shift per channel while gathering.
    """
    tok_gpos = aux["tok_gpos"]
    gamma, beta = aux["gamma"], aux["beta"]
    core = tok_gpos // L
    loc = tok_gpos % L

    nst = STAT_HI // BS
    S = np.zeros(D, dtype=np.float64)
    Q = np.zeros(D, dtype=np.float64)
    for c in range(NCORES):
        st = results[c]["st"].astype(np.float64)
        S += np.concatenate(
            [st[:, 0:nst].sum(axis=1), st[:, nst : 2 * nst].sum(axis=1)]
        )
        Q += np.concatenate(
            [st[:, 2 * nst : 3 * nst].sum(axis=1), st[:, 3 * nst :].sum(axis=1)]
        )
    valid = np.zeros((NCORES, L), dtype=bool)
    valid[core, loc] = True
    for c in range(NCORES):
        # device summed cols [0, STAT_HI) incl. gaps: subtract the gaps
        gap_cols = np.flatnonzero(~valid[c][:STAT_HI])
        yg = results[c]["out"][:, gap_cols].astype(np.float64)
        S -= yg.sum(axis=1)
        Q -= (yg * yg).sum(axis=1)
        # cols [STAT_HI, L) have no device stats: add their valid columns
        tail_cols = STAT_HI + np.flatnonzero(valid[c][STAT_HI:])
        yt = results[c]["out"][:, tail_cols].astype(np.float64)
        S += yt.sum(axis=1)
        Q += (yt * yt).sum(axis=1)

    mean = S / N
    var = Q / N - mean * mean
    scale = gamma.astype(np.float64) / np.sqrt(var + EPS)
    shift = beta.astype(np.float64) - mean * scale
    scale32 = scale.astype(np.float32)
    shift32 = shift.astype(np.float32)

    out = np.empty((N, D), dtype=np.float32)
    for c in range(NCORES):
        sel = core == c
        yc = results[c]["out"][:, loc[sel]].T.astype(np.float32)
        out[sel] = yc * scale32 + shift32
    return out


def kernel(x_all, W, b, gamma, beta, segment_key):
    nc = _get_program()
    in_maps, aux = prepare_inputs(x_all, W, b, gamma, beta, segment_key)
    res = run_bass_kernel_spmd(nc, in_maps, list(range(NCORES)))
    return assemble_output(res.results, aux)


# revision 39
# speedup vs baseline: 1.3636x; 1.3417x over previous
"""Trainium2 Bass kernel for segment-wise Conv1d + ReLU + BatchNorm1d.

Reference computation (nn_ConvSeg):
  - x_all [32768, 256] fp32, segment_key [32768] sorted ids (<= 8 segments)
  - per-segment Conv1d (kernel K=9, zero padding 4 at segment boundaries)
  - ReLU, then BatchNorm1d over all tokens (training stats, biased var)

Strategy:
  - Host inserts 4 zero rows at each segment boundary -> the ragged
    per-segment conv becomes ONE dense conv over the gapped sequence.
  - The gapped sequence (8*4104 positions) is split into 8 equal chunks
    (one per NeuronCore) with a 4-position halo on each side.
  - Data is transposed to [d, position] so each conv tap is a shifted
    column window of the same SBUF tile: conv = sum over (tap, d-chunk) of
    128x128 bf16 matmuls accumulated in fp32 PSUM ([d_out-chunk, pos]).
    bf16 inputs halve input DMA bytes at ~2.7e-3 rel err (tolerance 2e-2).
  - Blocks are 228 positions: measured on this hardware, matmuls with
    free dim <= ~228 stream ~2.6 cols/ns vs ~1.2 above ~256 - a 2x cliff
    (measured via pure-matmul count-differencing probes).
  - A few matmuls on a scratch tile run during the input-DMA head so the
    PE activity monitor un-throttles the clock before the real matmuls;
    a k=0 pre-pass over the first three blocks (which need only the first
    x chunk and the k0 weights) fills the PE's wait for the remaining
    weight DMAs.
  - ScalarE fuses bias + ReLU from PSUM and accumulates per-block column
    sums (accum_out); a second ScalarE pass accumulates sums of squares.
    Results DMA out (bf16) per ~900-column group as soon as ready,
    overlapping the remaining matmuls. Stats for the final group are
    computed on the host from the returned y, so the tiny stats DMA
    ships one group early and never sits on the critical tail.
  - The BatchNorm reduction across cores and the per-channel affine fold
    into the host-side unshard: the host subtracts the gap columns'
    contribution from the raw sums, reduces across the 8 cores, and
    applies y*scale+shift while reassembling [32768, 256]. No collective
    (the emulated-NRT AllReduce costs ~1 ms here), no second device pass.
"""

import numpy as np
import ml_dtypes

import concourse.bacc as bacc
import concourse.mybir as mybir
from concourse import tile
from concourse.bass_utils import run_bass_kernel_spmd

F32 = mybir.dt.float32
BF16 = mybir.dt.bfloat16
AF = mybir.ActivationFunctionType

N = 32768
D = 256  # d_in == d_out == 256
K = 9
PAD = K // 2
EPS = 1e-5

NCORES = 8
NB = 18  # matmul blocks per core
BS = 228  # positions per block (lives in the PE's fast free-dim regime)
L = NB * BS  # 4104 gapped positions per core
LH = L + 2 * PAD  # input columns incl. halo
GAP = 4  # zero rows inserted at each segment boundary (>= PAD)

# out-DMA column groups and x-DMA chunks (all boundaries are multiples of
# every supported block size)
OUTG = [(0, 912), (912, 1824), (1824, 2736), (2736, 3648), (3648, L)]
XCH = [(0, 692), (684, 920), (912, 2288), (2280, LH)]
# columns >= STAT_HI get their BN sums computed on the host (from the
# returned y) instead of on-device, so the stats DMA never sits on the
# critical tail - it only depends on the second-to-last output group
STAT_HI = 3648

_PROGRAM_CACHE: dict = {}


def build_program(repeat: int = 1, warm: int = 8, nb: int = None,
                  bs: int = None, out_bf16: bool = True):
    """Build + compile the SPMD Bass program (identical on all 8 cores)."""
    nb = NB if nb is None else nb
    bs = BS if bs is None else bs
    assert nb * bs == L
    ydt = BF16 if out_bf16 else F32
    nc = bacc.Bacc(
        "TRN2", target_bir_lowering=False, debug=False, num_devices=NCORES
    )

    nst = STAT_HI // bs  # blocks with on-device stats (per oc half)
    x_d = nc.declare_dram_parameter("x", [2, 128, LH], BF16, isOutput=False)
    w_d = nc.declare_dram_parameter("w", [2, 128, K * D], BF16, isOutput=False)
    b2_d = nc.declare_dram_parameter("b2", [128, 2], F32, isOutput=False)
    out_d = nc.declare_dram_parameter("out", [D, L], ydt, isOutput=True)
    st_d = nc.declare_dram_parameter("st", [128, 4 * nst], F32, isOutput=True)

    with tile.TileContext(nc) as tc:
        with (
            tc.tile_pool(name="const", bufs=1) as const,
            tc.tile_pool(name="ypool", bufs=1) as ypool,
            tc.tile_pool(name="psum", bufs=6, space="PSUM") as psum,
            tc.tile_pool(name="pswarm", bufs=1, space="PSUM") as pswarm,
            tc.tile_pool(name="work", bufs=2) as work,
            tc.tile_pool(name="stats", bufs=1) as stats,
        ):
            xt = [const.tile([128, LH], BF16, tag=f"xt{dc}", name=f"xt{dc}")
                  for dc in range(2)]
            wt = [const.tile([128, K * D], BF16, tag=f"wt{dc}", name=f"wt{dc}")
                  for dc in range(2)]
            b2t = const.tile([128, 2], F32)
            # scratch warmup operand: never written, contents irrelevant
            wz = const.tile([128, 464], BF16, tag="wz", name="wz")
            ybig = ypool.tile([128, 2 * L], ydt)
            # per-block raw sums: cols [0,2nst) = sum(y), rest = sum(y^2)
            stq = stats.tile([128, 4 * nst], F32)

            if warm:
                nc.gpsimd.memset(wz[:], 0.0)

            for _ in range(repeat):
                # --- PE warmup: no data deps, runs during the DMA head so
                # the activity monitor un-throttles the clock ---
                if warm:
                    psw = pswarm.tile([128, min(bs, 464)], F32, tag="psw")
                    for _ in range(warm):
                        nc.tensor.matmul(
                            psw[:], wz[:, 0:128], wz[:, 0 : min(bs, 464)],
                            start=True, stop=True,
                        )

                # --- input DMAs, ordered to match PE consumption times ---
                for dc in range(2):  # tap k=0 weights
                    nc.sync.dma_start(wt[dc][:, 0:D], w_d[dc, :, 0:D])
                for dc in range(2):  # first x chunk
                    lo, hi = XCH[0]
                    nc.sync.dma_start(xt[dc][:, lo:hi], x_d[dc, :, lo:hi])
                # remaining weights, dc-major halves so arrival order
                # matches the first group's dc-major consumption order
                for dc in range(2):
                    nc.sync.dma_start(
                        wt[dc][:, D : 5 * D], w_d[dc, :, D : 5 * D]
                    )
                    nc.sync.dma_start(wt[dc][:, 5 * D :], w_d[dc, :, 5 * D :])
                for dc in range(2):  # second x chunk
                    lo, hi = XCH[1]
                    nc.sync.dma_start(xt[dc][:, lo:hi], x_d[dc, :, lo:hi])
                nc.sync.dma_start(b2t[:], b2_d[:])  # needed by first relu
                for ch in range(2, len(XCH)):
                    lo, hi = XCH[ch]
                    for dc in range(2):
                        nc.sync.dma_start(xt[dc][:, lo:hi], x_d[dc, :, lo:hi])

                # --- conv + relu(+bias) + raw stats + streaming out-DMA ---
                # k=0 pre-pass over the first blocks: needs only the first
                # x chunk + k0 weights, which land ~1.4us before the rest
                # of the weights - fills the PE's wait for that DMA
                nA = (XCH[0][1] - 2 * PAD) // bs
                pre = {}
                for b in range(nA):
                    for oc in range(2):
                        ps = psum.tile([128, bs], F32, tag="ps")
                        pre[(b, oc)] = ps
                        for dc in range(2):
                            nc.tensor.matmul(
                                ps[:],
                                wt[dc][:, oc * 128 : oc * 128 + 128],
                                xt[dc][:, b * bs : b * bs + bs],
                                start=(dc == 0), stop=False,
                            )
                for glo, ghi in OUTG:
                    for b in range(glo // bs, ghi // bs):
                        for oc in range(2):
                            if (b, oc) in pre:
                                ps = pre[(b, oc)]
                                k0 = 1
                            else:
                                ps = psum.tile([128, bs], F32, tag="ps")
                                k0 = 0
                            # dc-major so the dc=0 taps can run while the
                            # dc=1 weight DMA is still in flight early on
                            for dc in range(2):
                                for k in range(k0, K):
                                    nc.tensor.matmul(
                                        ps[:],
                                        wt[dc][
                                            :, k * D + oc * 128
                                            : k * D + oc * 128 + 128
                                        ],
                                        xt[dc][:, b * bs + k : b * bs + k + bs],
                                        start=(k0 == 0 and k == 0 and dc == 0),
                                        stop=(k == K - 1 and dc == 1),
                                    )
                            ysl = ybig[:, oc * L + b * bs : oc * L + (b + 1) * bs]
                            if b < nst:
                                j = oc * nst + b
                                # y = relu(conv + bias); accum_out = sum(y)
                                nc.scalar.activation(
                                    ysl, ps[:], AF.Relu,
                                    bias=b2t[:, oc : oc + 1], scale=1.0,
                                    accum_out=stq[:, j : j + 1],
                                )
                                # sum of squares via a second ScalarE pass
                                # (tensor_tensor_reduce crashes the device)
                                sq = work.tile([128, bs], F32, tag="sq")
                                nc.scalar.activation(
                                    sq[:], ysl, AF.Square, bias=0.0,
                                    scale=1.0,
                                    accum_out=stq[:, 2 * nst + j
                                                  : 2 * nst + j + 1],
                                )
                            else:
                                # host computes this tail region's stats
                                nc.scalar.activation(
                                    ysl, ps[:], AF.Relu,
                                    bias=b2t[:, oc : oc + 1], scale=1.0,
                                )
                    for oc in range(2):
                        nc.sync.dma_start(
                            out_d[oc * 128 : (oc + 1) * 128, glo:ghi],
                            ybig[:, oc * L + glo : oc * L + ghi],
                        )
                    if ghi == STAT_HI:
                        # all on-device stats complete; ship them now so
                        # this DMA hides under the last compute group
                        nc.sync.dma_start(st_d[:], stq[:])

    nc.compile()
    return nc


def _get_program(repeat: int = 1):
    key = repeat
    if key not in _PROGRAM_CACHE:
        _PROGRAM_CACHE[key] = build_program(repeat)
    return _PROGRAM_CACHE[key]


def prepare_inputs(x_all, W, b, gamma, beta, segment_key):
    """Host-side sharding: gap insertion, transpose, per-core slicing.

    Returns (in_maps, aux); aux carries everything assemble_output needs.
    """
    x_all = np.ascontiguousarray(np.asarray(x_all, dtype=np.float32))
    W = np.asarray(W, dtype=np.float32)
    b = np.asarray(b, dtype=np.float32)
    gamma = np.asarray(gamma, dtype=np.float32)
    beta = np.asarray(beta, dtype=np.float32)
    seg = np.asarray(segment_key).reshape(-1)
    n = x_all.shape[0]
    assert n == N, f"kernel hardcodes N={N}, got {n}"

    # run-length segments of the sorted key
    change = np.flatnonzero(seg[1:] != seg[:-1]) + 1
    starts = np.concatenate(([0], change))
    ends = np.concatenate((change, [n]))
    nseg = len(starts)
    assert n + GAP * (nseg + 1) <= NCORES * L, "gapped sequence does not fit"

    # gapped position of each token
    tok_gpos = np.empty(n, dtype=np.int64)
    g = GAP
    for s, e in zip(starts, ends):
        tok_gpos[s:e] = g + np.arange(e - s)
        g += (e - s) + GAP

    # gapped, transposed input with halo: xg_t[:, PAD + gpos] = x_all[n]
    total = NCORES * L
    xg = np.zeros((total + 2 * PAD, D), dtype=np.float32)
    xg[PAD + tok_gpos] = x_all
    xg_t = np.ascontiguousarray(xg.T.astype(ml_dtypes.bfloat16))

    # weights: wmat[d, k*D + o] = W[o, d, k]
    wmat = W.transpose(1, 2, 0).reshape(D, K * D).astype(ml_dtypes.bfloat16)
    w_in = np.ascontiguousarray(wmat.reshape(2, 128, K * D))

    b2 = np.ascontiguousarray(np.stack([b[:128], b[128:]], axis=1))

    in_maps = []
    for c in range(NCORES):
        xc = np.ascontiguousarray(
            xg_t[:, c * L : c * L + LH].reshape(2, 128, LH)
        )
        in_maps.append({"x": xc, "w": w_in, "b2": b2})
    aux = {"tok_gpos": tok_gpos, "gamma": gamma, "beta": beta}
    return in_maps, aux


def assemble_output(results, aux):
    """Unshard + fold the BatchNorm affine.

    Device sums include the gap columns; subtract their contribution (from
    the very same f32 y values the device summed), reduce across cores,
    then apply y*scale + shift per channel while gathering.
    """
    tok_gpos = aux["tok_gpos"]
    gamma, beta = aux["gamma"], aux["beta"]
    core = tok_gpos // L
    loc = tok_gpos % L

    nst = STAT_HI // BS
    S = np.zeros(D, dtype=np.float64)
    Q = np.zeros(D, dtype=np.float64)
    for c in range(NCORES):
        st = results[c]["st"].astype(np.float64)
        S += np.concatenate(
            [st[:, 0:nst].sum(axis=1), st[:, nst : 2 * nst].sum(axis=1)]
        )
        Q += np.concatenate(
            [st[:, 2 * nst : 3 * nst].sum(axis=1), st[:, 3 * nst :].sum(axis=1)]
        )
    valid = np.zeros((NCORES, L), dtype=bool)
    valid[core, loc] = True
    for c in range(NCORES):
        # device summed cols [0, STAT_HI) incl. gaps: subtract the gaps
        gap_cols = np.flatnonzero(~valid[c][:STAT_HI])
        yg = results[c]["out"][:, gap_cols].astype(np.float64)
        S -= yg.sum(axis=1)
        Q -= (yg * yg).sum(axis=1)
        # cols [STAT_HI, L) have no device stats: add their valid columns
        tail_cols = STAT_HI + np.flatnonzero(valid[c][STAT_HI:])
        yt = results[c]["out"][:, tail_cols].astype(np.float64)
        S += yt.sum(axis=1)
        Q += (yt * yt).sum(axis=1)

    mean = S / N
    var = Q / N - mean * mean
    scale = gamma.astype(np.float64) / np.sqrt(var + EPS)
    shift = beta.astype(np.float64) - mean * scale
    scale32 = scale.astype(np.float32)
    shift32 = shift.astype(np.float32)

    out = np.empty((N, D), dtype=np.float32)
    for c in range(NCORES):
        sel = core == c
        yc = results[c]["out"][:, loc[sel]].T.astype(np.float32)
        out[sel] = yc * scale32 + shift32
    return out


def kernel(x_all, W, b, gamma, beta, segment_key):
    nc = _get_program()
    in_maps, aux = prepare_inputs(x_all, W, b, gamma, beta, segment_key)
    res = run_bass_kernel_spmd(nc, in_maps, list(range(NCORES)))
    return assemble_output(res.results, aux)
